# revision 1
# baseline (speedup 1.0000x reference)
"""AccumulateNeighbours (meanmax) Trainium2 kernel.

out[v] = concat(mean_k feat[nidx[v,k]], max_k feat[nidx[v,k]])  -> [V, 2F]

Strategy: shard vertices (rows of feat/nidx) across the 8 NeuronCores; the
feat table is replicated into each core's HBM so the gather is fully local
and data-parallel. Per core, loop over 128-vertex tiles:
  1. DMA the tile's neighbour indices [128, K] into SBUF (HWDGE).
  2. K per-partition indirect DMAs (SWDGE) gather the neighbour rows into
     a [128, K, F] SBUF tile -- the HW DGE supports one dynamic offset per
     partition per instruction, so each instruction fetches 128 rows.
  3. Vector-engine reductions over K produce mean (via sum * 1/K on the
     scalar engine) and max; results land in a [128, 2F] tile.
  4. DMA the output tile back to HBM.
Tile pools double-buffer everything so gathers stream back-to-back; the
kernel is bound by SWDGE descriptor generation on the GpSimd Q7 engine
(~8.5ns/descriptor + ~320ns dispatch per instruction, measured).

int64 nidx is handled zero-copy: the little-endian low words are read with
a stride-2 int32 access pattern.
"""

import numpy as np

import concourse.bacc as bacc
import concourse.bass as bass
import concourse.mybir as mybir
import concourse.tile as tile
from concourse import bass_utils

V, K, F = 150000, 32, 96
NCORES = 8
VS = V // NCORES  # 18750 vertices per core
P = 128

GATHER_BUFS = 4
IDX_BUFS = 4
OUT_BUFS = 4

_prog_cache: dict = {}


def _build(idx_cols: int, idx_step: int):
    nc = bacc.Bacc("TRN2", target_bir_lowering=False, debug=False)
    feat_d = nc.dram_tensor("feat", [V, F], mybir.dt.float32, kind="ExternalInput")
    nidx_d = nc.dram_tensor("nidx", [VS, idx_cols], mybir.dt.int32, kind="ExternalInput")
    out_d = nc.dram_tensor("out", [VS, 2 * F], mybir.dt.float32, kind="ExternalOutput")

    feat_ap = feat_d.ap()
    nidx_ap = nidx_d.ap()
    out_ap = out_d.ap()

    ntiles = (VS + P - 1) // P
    with tile.TileContext(nc) as tc:
        with (
            tc.tile_pool(name="idx", bufs=IDX_BUFS) as idx_pool,
            tc.tile_pool(name="gather", bufs=GATHER_BUFS) as g_pool,
            tc.tile_pool(name="sum", bufs=2) as s_pool,
            tc.tile_pool(name="out", bufs=OUT_BUFS) as o_pool,
        ):
            for t in range(ntiles):
                rows = min(P, VS - t * P)
                idx_tile = idx_pool.tile([P, idx_cols], mybir.dt.int32)
                nc.sync.dma_start(
                    idx_tile[:rows, :], nidx_ap[t * P : t * P + rows, :]
                )
                g_tile = g_pool.tile([P, K, F], mybir.dt.float32)
                for j in range(K):
                    nc.gpsimd.indirect_dma_start(
                        out=g_tile[:rows, j],
                        out_offset=None,
                        in_=feat_ap[:, :],
                        in_offset=bass.IndirectOffsetOnAxis(
                            ap=idx_tile[:rows, j * idx_step : j * idx_step + 1],
                            axis=0,
                        ),
                    )
                perm = g_tile[:rows].rearrange("p k f -> p f k")
                sum_tile = s_pool.tile([P, F], mybir.dt.float32)
                nc.vector.reduce_sum(sum_tile[:rows], perm, axis=mybir.AxisListType.X)
                o_tile = o_pool.tile([P, 2 * F], mybir.dt.float32)
                nc.scalar.mul(o_tile[:rows, 0:F], sum_tile[:rows], 1.0 / K)
                nc.vector.reduce_max(
                    o_tile[:rows, F : 2 * F], perm, axis=mybir.AxisListType.X
                )
                nc.sync.dma_start(out_ap[t * P : t * P + rows, :], o_tile[:rows, :])
    nc.compile()
    return nc


def _get_prog(idx_cols, idx_step):
    key = (idx_cols, idx_step, GATHER_BUFS)
    if key not in _prog_cache:
        _prog_cache[key] = _build(idx_cols, idx_step)
    return _prog_cache[key]


def kernel(feat: np.ndarray, nidx: np.ndarray, **run_kwargs):
    assert feat.shape == (V, F), feat.shape
    assert nidx.shape == (V, K), nidx.shape
    feat = np.ascontiguousarray(feat, dtype=np.float32)
    if nidx.dtype == np.int64:
        nidx = np.ascontiguousarray(nidx)
        nidx32 = nidx.view(np.int32)  # [V, 2K]; low word = value (LE)
        idx_cols, idx_step = 2 * K, 2
    else:
        nidx32 = np.ascontiguousarray(nidx.astype(np.int32, copy=False))
        idx_cols, idx_step = K, 1

    nc = _get_prog(idx_cols, idx_step)
    in_maps = [
        {"feat": feat, "nidx": nidx32[c * VS : (c + 1) * VS]} for c in range(NCORES)
    ]
    res = bass_utils.run_bass_kernel_spmd(
        nc, in_maps, core_ids=list(range(NCORES)), **run_kwargs
    )
    out = np.concatenate([res.results[c]["out"] for c in range(NCORES)], axis=0)
    if run_kwargs:
        return out, res
    return out



# revision 5
# speedup vs baseline: 1.0078x; 1.0078x over previous
"""AccumulateNeighbours (meanmax) Trainium2 kernel.

out[v] = concat(mean_k feat[nidx[v,k]], max_k feat[nidx[v,k]])  -> [V, 2F]

Strategy: shard vertices (rows of feat/nidx) across the 8 NeuronCores; the
feat table is replicated into each core's HBM so the gather is fully local
and data-parallel. Per core, loop over 128-vertex tiles:
  1. DMA the tile's neighbour indices [128, K] into SBUF (HWDGE).
  2. K per-partition indirect DMAs (SWDGE) gather the neighbour rows into
     a [128, K, F] SBUF tile -- the HW DGE supports one dynamic offset per
     partition per instruction, so each instruction fetches 128 rows.
  3. Vector-engine reductions over K produce mean (via sum * 1/K on the
     scalar engine) and max; results land in a [128, 2F] tile.
  4. DMA the output tile back to HBM.
Tile pools double-buffer everything so gathers stream back-to-back; the
kernel is bound by SWDGE descriptor generation on the GpSimd Q7 engine
(~8.5ns/descriptor + ~320ns dispatch per instruction, measured).

int64 nidx is handled zero-copy: the little-endian low words are read with
a stride-2 int32 access pattern.
"""

import numpy as np

import concourse.bacc as bacc
import concourse.bass as bass
import concourse.mybir as mybir
import concourse.tile as tile
from concourse import bass_utils

V, K, F = 150000, 32, 96
NCORES = 8
VS = V // NCORES  # 18750 vertices per core
P = 128

GATHER_BUFS = 4
IDX_BUFS = 4
OUT_BUFS = 4

_prog_cache: dict = {}


NQUEUES = 4


def _build(idx_cols: int, idx_step: int):
    nc = bacc.Bacc(
        "TRN2", target_bir_lowering=False, debug=False, num_swdge_queues=NQUEUES
    )
    feat_d = nc.dram_tensor("feat", [V, F], mybir.dt.float32, kind="ExternalInput")
    nidx_d = nc.dram_tensor("nidx", [VS, idx_cols], mybir.dt.int32, kind="ExternalInput")
    out_d = nc.dram_tensor("out", [VS, 2 * F], mybir.dt.float32, kind="ExternalOutput")

    feat_ap = feat_d.ap()
    nidx_ap = nidx_d.ap()
    out_ap = out_d.ap()

    ntiles = (VS + P - 1) // P
    with tile.TileContext(nc) as tc:
        with (
            tc.tile_pool(name="idx", bufs=IDX_BUFS) as idx_pool,
            tc.tile_pool(name="gather", bufs=GATHER_BUFS) as g_pool,
            tc.tile_pool(name="sum", bufs=2) as s_pool,
            tc.tile_pool(name="out", bufs=OUT_BUFS) as o_pool,
        ):
            for t in range(ntiles):
                rows = min(P, VS - t * P)
                idx_tile = idx_pool.tile([P, idx_cols], mybir.dt.int32)
                nc.sync.dma_start(
                    idx_tile[:rows, :], nidx_ap[t * P : t * P + rows, :]
                )
                g_tile = g_pool.tile([P, K, F], mybir.dt.float32)
                for j in range(K):
                    inst = nc.gpsimd.indirect_dma_start(
                        out=g_tile[:rows, j],
                        out_offset=None,
                        in_=feat_ap[:, :],
                        in_offset=bass.IndirectOffsetOnAxis(
                            ap=idx_tile[:rows, j * idx_step : j * idx_step + 1],
                            axis=0,
                        ),
                    )
                    # Round-robin the SWDGE queues: descriptor generation for
                    # queue q runs on Q7 core pair q, so 4 queues generate
                    # descriptors in parallel (the serial Q7 desc-gen is the
                    # baseline bottleneck).
                    q = j % NQUEUES
                    if q:
                        inst.queue = f"qPoolDynamic{q}"
                perm = g_tile[:rows].rearrange("p k f -> p f k")
                sum_tile = s_pool.tile([P, F], mybir.dt.float32)
                nc.vector.reduce_sum(sum_tile[:rows], perm, axis=mybir.AxisListType.X)
                o_tile = o_pool.tile([P, 2 * F], mybir.dt.float32)
                nc.scalar.mul(o_tile[:rows, 0:F], sum_tile[:rows], 1.0 / K)
                nc.vector.reduce_max(
                    o_tile[:rows, F : 2 * F], perm, axis=mybir.AxisListType.X
                )
                nc.sync.dma_start(out_ap[t * P : t * P + rows, :], o_tile[:rows, :])
    nc.compile()
    return nc


def _get_prog(idx_cols, idx_step):
    key = (idx_cols, idx_step, GATHER_BUFS)
    if key not in _prog_cache:
        _prog_cache[key] = _build(idx_cols, idx_step)
    return _prog_cache[key]


def kernel(feat: np.ndarray, nidx: np.ndarray, **run_kwargs):
    assert feat.shape == (V, F), feat.shape
    assert nidx.shape == (V, K), nidx.shape
    feat = np.ascontiguousarray(feat, dtype=np.float32)
    if nidx.dtype == np.int64:
        nidx = np.ascontiguousarray(nidx)
        nidx32 = nidx.view(np.int32)  # [V, 2K]; low word = value (LE)
        idx_cols, idx_step = 2 * K, 2
    else:
        nidx32 = np.ascontiguousarray(nidx.astype(np.int32, copy=False))
        idx_cols, idx_step = K, 1

    nc = _get_prog(idx_cols, idx_step)
    in_maps = [
        {"feat": feat, "nidx": nidx32[c * VS : (c + 1) * VS]} for c in range(NCORES)
    ]
    res = bass_utils.run_bass_kernel_spmd(
        nc, in_maps, core_ids=list(range(NCORES)), **run_kwargs
    )
    out = np.concatenate([res.results[c]["out"] for c in range(NCORES)], axis=0)
    if run_kwargs:
        return out, res
    return out

